# revision 23
# baseline (speedup 1.0000x reference)
"""Block-local attention + LayerNorm kernel for Trainium2 (8 NeuronCores).

Problem (see reference):
  inputs [B=4, bn=16, bl=512, dim=512] fp32
  Q = X@W1, K = X@W2, V = X@W3 (+zero biases)
  S = Q K^T / sqrt(512), masked by elementwise {0,1} mask, softmax over keys
  out = LayerNorm(P @ V + X, eps=1e-3)

Sharding: 64 independent (batch, block) pairs -> 8 blocks per core.

TRN2 measurement: every PE matmul instruction costs ~310 ns regardless
of dtype/mode (fp8 DoubleRow covers K=256 per instr, f32r/bf16 K=128),
so the design minimizes PE instruction count (48/block):
  A^T  : 8  fp8 DoubleRow MMs   (w12 = fp8(256*W1W2^T/sqrt(d)),
                                 xt8 = fp8(X^T/2); PSUM = 128*A^T)
  V    : 8  fp8 DoubleRow MMs   (w3 = fp8(2*W3); PSUM = V exactly)
  S^T  : 16 bf16 MMs            (xtb = bf16(X^T), at = bf16(A^T) via
                                 ACT-copy scale=1/128; PSUM = S^T)
  rowsm: 8  tiny fp8 DR MMs     (r[q] = sum_k P_u^T[k,q], ones rhs)
  O    : 8  fp8 DoubleRow MMs   (P_u^T fp8 x V fp8)
The mask never touches the PE: maskbias^T (0/-240, fp8) is DMA'd
straight into the S^T PSUM tile by a gpsimd casting DMA before the
matmuls accumulate onto it (start=False), so one ACT op per chunk does
exp(S^T + maskbias - 2) -> P_u^T fp8.  Softmax normalization + residual
use LayerNorm scale invariance:
  z = P_u V + r * X  (DVE scalar_tensor_tensor, f32)   LN(z) w/ eps*r^2
so there is no reciprocal or normalize pass over P.
"""

import math
import sys

import numpy as np

sys.path.insert(0, "/opt/trn_rl_repo")

import ml_dtypes

import concourse.bacc as bacc
import concourse.tile as tile
from concourse import masks, mybir
from concourse.bass_utils import run_bass_kernel_spmd

DIM = 512
BLOCK_NUM = 16
SEQ_LEN = 8192
BLOCK_LEN = 512
BATCH = 4
LN_EPS = 1e-3
N_CORES = 8
NBLK = (BATCH * BLOCK_NUM) // N_CORES  # blocks per core
NC_P = 128  # partitions
NCH = DIM // NC_P  # 4 chunks of 128 along dim/token axes

F32 = mybir.dt.float32
BF16 = mybir.dt.bfloat16
F8 = mybir.dt.float8e4
I32 = mybir.dt.int32
DR = mybir.MatmulPerfMode.DoubleRow
EXP = mybir.ActivationFunctionType.Exp
IDENT_FN = mybir.ActivationFunctionType.Identity
MUL = mybir.AluOpType.mult
ADD = mybir.AluOpType.add

NP_F8 = ml_dtypes.float8_e4m3
NP_BF16 = ml_dtypes.bfloat16

MASK_NEG = -240.0  # fp8-max-finite additive mask bias; exp(S-240-2) == 0
EXP_BIAS = -2.0  # headroom shift; cancels exactly in the normalization
C_X = 2.0  # xt8 = X / 2
C_W12 = 256.0  # w12 = 256 * (W1 W2^T/sqrt(d)); A_psum = 128*A
AT_SCALE = 1.0 / 128.0  # at = bf16(A_psum/128) = A


def build_nc(nblk=NBLK, repeat=1):
    nc = bacc.Bacc("TRN2", target_bir_lowering=False, debug=False,
                   num_devices=N_CORES)

    xt8_d = nc.declare_dram_parameter("xt8", [nblk, NC_P, NCH, DIM], F8, isOutput=False)
    xtb_d = nc.declare_dram_parameter("xtb", [nblk, NC_P, NCH, DIM], BF16, isOutput=False)
    xn_d = nc.declare_dram_parameter("xn", [nblk, NC_P, NCH, DIM], BF16, isOutput=False)
    mk_d = nc.declare_dram_parameter("mk", [nblk, NC_P, NCH, DIM], BF16, isOutput=False)
    w12_d = nc.declare_dram_parameter("w12", [NC_P, NCH, DIM], F8, isOutput=False)
    w3_d = nc.declare_dram_parameter("w3", [NC_P, NCH, DIM], F8, isOutput=False)
    out_d = nc.declare_dram_parameter("out", [nblk, NC_P, NCH, DIM], F32, isOutput=True)

    with tile.TileContext(nc) as tc:
        with (
            tc.tile_pool(name="const", bufs=1) as const,
            tc.tile_pool(name="xt8", bufs=3) as p_xt8,
            tc.tile_pool(name="xtb", bufs=2) as p_xtb,
            tc.tile_pool(name="xn", bufs=2) as p_xn,
            tc.tile_pool(name="mk", bufs=2) as p_mk,
            tc.tile_pool(name="at", bufs=2) as p_at,
            tc.tile_pool(name="v", bufs=2) as p_v,
            tc.tile_pool(name="pt", bufs=2) as p_pt,
            tc.tile_pool(name="z", bufs=2) as p_z,
            tc.tile_pool(name="ob", bufs=2) as p_ob,
            tc.tile_pool(name="tiny", bufs=4) as p_tiny,
            tc.tile_pool(name="ps_mm", bufs=3, space="PSUM") as ps_mm,
            tc.tile_pool(name="ps_o", bufs=3, space="PSUM") as ps_o,
            tc.tile_pool(name="ps_r", bufs=1, space="PSUM") as ps_r,
        ):
            # persistent constants
            w12_sb = const.tile([NC_P, NCH, DIM], F8)
            nc.sync.dma_start(out=w12_sb, in_=w12_d[:])
            w3_sb = const.tile([NC_P, NCH, DIM], F8)
            nc.gpsimd.dma_start(out=w3_sb, in_=w3_d[:])
            ones2 = const.tile([NC_P, 2, 1], F8)
            nc.vector.memset(ones2, 1.0)
            ebias = const.tile([NC_P, 1], F32)
            nc.vector.memset(ebias, EXP_BIAS)
            ident = const.tile([NC_P, NC_P], F32)
            masks.make_identity(nc, ident[:])
            identb = const.tile([NC_P, NC_P], BF16)
            nc.vector.tensor_copy(identb[:], ident[:])

            def _blocks():
              for b in range(nblk):
                xt8_sb = p_xt8.tile([NC_P, NCH, DIM], F8, tag="xt8")
                nc.sync.dma_start(out=xt8_sb, in_=xt8_d[b])
                xtb_sb = p_xtb.tile([NC_P, NCH, DIM], BF16, tag="xtb")
                nc.sync.dma_start(out=xtb_sb, in_=xtb_d[b])
                xn_sb = p_xn.tile([NC_P, NCH, DIM], BF16, tag="xn")
                nc.gpsimd.dma_start(out=xn_sb, in_=xn_d[b])
                mk_sb = p_mk.tile([NC_P, NCH, DIM], BF16, tag="mk")
                nc.scalar.dma_start(out=mk_sb, in_=mk_d[b])

                # at[d2, t] = bf16(A^T): PSUM = 128*A^T, ACT copy * 1/128
                at_sb = p_at.tile([NC_P, NCH, DIM], BF16, tag="at")
                for d2c in range(NCH):
                    ps = ps_mm.tile([NC_P, DIM], F32, tag="mm")
                    for i in range(2):
                        nc.tensor.matmul(
                            ps[:],
                            lhsT=w12_sb[:, 2 * i:2 * i + 2,
                                        d2c * NC_P:(d2c + 1) * NC_P],
                            rhs=xt8_sb[:, 2 * i:2 * i + 2, :],
                            start=(i == 0), stop=(i == 1), perf_mode=DR)
                    nc.scalar.mul(at_sb[:, d2c, :], ps[:], AT_SCALE)

                # v[t, d'] = fp8(V) = (X^T/2) (2 W3)
                v_sb = p_v.tile([NC_P, NCH, DIM], F8, tag="v")
                for tc_i in range(NCH):
                    ps = ps_mm.tile([NC_P, DIM], F32, tag="mm")
                    for i in range(2):
                        nc.tensor.matmul(
                            ps[:],
                            lhsT=xt8_sb[:, 2 * i:2 * i + 2,
                                        tc_i * NC_P:(tc_i + 1) * NC_P],
                            rhs=w3_sb[:, 2 * i:2 * i + 2, :],
                            start=(i == 0), stop=(i == 1), perf_mode=DR)
                    if tc_i < 2:
                        nc.scalar.copy(v_sb[:, tc_i, :], ps[:])
                    else:
                        nc.vector.tensor_copy(v_sb[:, tc_i, :], ps[:])

                # S^T per key-chunk: PSUM seeded with maskbias^T by a
                # casting DMA, then 4 bf16 MMs accumulate S^T on top;
                # pt = exp(S^T + maskbias - 2) in fp8 (one ACT op each)
                pt_sb = p_pt.tile([NC_P, NCH, DIM], F8, tag="pt")
                for kc in range(NCH):
                    ps = ps_mm.tile([NC_P, DIM], F32, tag="mm")
                    for dc in range(NCH):
                        nc.tensor.matmul(
                            ps[:],
                            lhsT=xtb_sb[:, dc, kc * NC_P:(kc + 1) * NC_P],
                            rhs=at_sb[:, dc, :],
                            start=(dc == 0), stop=False)
                    # mask seeding last (bf16, same PE mode as the S MMs):
                    # S-MMs need not wait for the mk DMA
                    nc.tensor.matmul(ps[:], lhsT=identb[:],
                                     rhs=mk_sb[:, kc, :],
                                     start=False, stop=True)
                    nc.scalar.activation(pt_sb[:, kc, :], ps[:], EXP,
                                         bias=ebias[:])

                # rowsums r[q] = sum_k P_u^T[k, q] via ones-matmuls
                psr = ps_r.tile([NC_P, NCH], F32, tag="r")
                for qc in range(NCH):
                    for i in range(2):
                        nc.tensor.matmul(
                            psr[:, qc:qc + 1],
                            lhsT=pt_sb[:, 2 * i:2 * i + 2,
                                       qc * NC_P:(qc + 1) * NC_P],
                            rhs=ones2[:],
                            start=(i == 0), stop=(i == 1), perf_mode=DR)
                r_sb = p_tiny.tile([NC_P, NCH], F32, tag="r")
                nc.vector.tensor_copy(r_sb[:], psr[:])

                # z = P_u V + r * X  (= r * (attn + X), LN-scale-inv.)
                mvb = p_tiny.tile([NC_P, NCH, 2], F32, tag="mvb")
                z_sb = p_z.tile([NC_P, NCH, DIM], F32, tag="z")
                for qc in range(NCH):
                    pso = ps_o.tile([NC_P, DIM], F32, tag="o")
                    for i in range(2):
                        nc.tensor.matmul(
                            pso[:],
                            lhsT=pt_sb[:, 2 * i:2 * i + 2,
                                       qc * NC_P:(qc + 1) * NC_P],
                            rhs=v_sb[:, 2 * i:2 * i + 2, :],
                            start=(i == 0), stop=(i == 1), perf_mode=DR)
                    nc.vector.scalar_tensor_tensor(
                        out=z_sb[:, qc, :], in0=xn_sb[:, qc, :],
                        scalar=r_sb[:, qc:qc + 1], in1=pso[:],
                        op0=MUL, op1=ADD)
                    stats = p_tiny.tile([NC_P, 6], F32, tag="st")
                    nc.vector.bn_stats(stats[:], z_sb[:, qc, :])
                    nc.vector.bn_aggr(mvb[:, qc, :], stats[:])

                # istd = rsqrt(var_z + eps*r^2), batched over the 4 chunks
                # (magic-constant + 2 Newton steps, DVE only)
                rr = p_tiny.tile([NC_P, NCH], F32, tag="rr")
                nc.vector.tensor_mul(rr[:], r_sb[:], r_sb[:])
                tv = p_tiny.tile([NC_P, NCH], F32, tag="tv")
                nc.vector.scalar_tensor_tensor(
                    out=tv[:], in0=rr[:], scalar=LN_EPS, in1=mvb[:, :, 1],
                    op0=MUL, op1=ADD)
                yv = p_tiny.tile([NC_P, NCH], F32, tag="yv")
                hv = p_tiny.tile([NC_P, NCH], F32, tag="hv")
                nc.vector.tensor_scalar(
                    out=hv[:].bitcast(I32), in0=tv[:].bitcast(I32),
                    scalar1=1, scalar2=None,
                    op0=mybir.AluOpType.logical_shift_right)
                nc.vector.tensor_scalar(
                    out=yv[:].bitcast(I32), in0=hv[:].bitcast(I32),
                    scalar1=-1, scalar2=0x5F3759DF,
                    op0=MUL, op1=ADD)
                av = p_tiny.tile([NC_P, NCH], F32, tag="av")
                cv = p_tiny.tile([NC_P, NCH], F32, tag="cv")
                for _ in range(2):
                    nc.vector.tensor_mul(av[:], yv[:], yv[:])
                    nc.vector.tensor_mul(av[:], av[:], tv[:])
                    nc.vector.tensor_scalar(
                        out=cv[:], in0=av[:], scalar1=-0.5, scalar2=1.5,
                        op0=MUL, op1=ADD)
                    nc.vector.tensor_mul(yv[:], yv[:], cv[:])
                negms = p_tiny.tile([NC_P, NCH], F32, tag="negms")
                nc.vector.tensor_mul(negms[:], mvb[:, :, 0], yv[:])
                nc.vector.tensor_scalar_mul(negms[:], negms[:], -1.0)

                ob_sb = p_ob.tile([NC_P, NCH, DIM], F32, tag="ob")
                for qc in range(NCH):
                    if qc < 2:
                        nc.scalar.activation(
                            ob_sb[:, qc, :], z_sb[:, qc, :], IDENT_FN,
                            bias=negms[:, qc:qc + 1],
                            scale=yv[:, qc:qc + 1])
                    else:
                        nc.vector.tensor_scalar(
                            out=ob_sb[:, qc, :], in0=z_sb[:, qc, :],
                            scalar1=yv[:, qc:qc + 1],
                            scalar2=negms[:, qc:qc + 1],
                            op0=MUL, op1=ADD)
                nc.gpsimd.dma_start(out=out_d[b], in_=ob_sb[:])

            if repeat == 1:
                _blocks()
            else:
                with tc.For_i(0, repeat, 1):
                    _blocks()

    nc.finalize()
    return nc


_NC_CACHE = {}


def _get_nc():
    if "nc" not in _NC_CACHE:
        _NC_CACHE["nc"] = build_nc()
    return _NC_CACHE["nc"]


def prep_in_maps(inputs, mask_array, dw1, dw2, dw3, db1, db2, db3):
    X = np.ascontiguousarray(
        np.asarray(inputs, dtype=np.float32).reshape(
            BATCH * BLOCK_NUM, BLOCK_LEN, DIM))
    m = np.asarray(mask_array, dtype=np.float32).reshape(
        BATCH * BLOCK_NUM, BLOCK_LEN, DIM)
    nb = BATCH * BLOCK_NUM

    # xt[b,p,c,t] = X[b,t,c*128+p]  (X^T in partition-chunk order)
    xt = np.ascontiguousarray(
        X.reshape(nb, BLOCK_LEN, NCH, NC_P).transpose(0, 3, 2, 1))
    xt8 = (xt * np.float32(1.0 / C_X)).astype(NP_F8)
    xtb = xt.astype(NP_BF16)
    # xn[b,p,c,d] = X[b,c*128+p,d]  (bf16, natural rows; b3 folded in)
    xn_nat = X.reshape(nb, NCH, NC_P, DIM).transpose(0, 2, 1, 3)
    db3 = np.asarray(db3, np.float32)
    if db3.any():
        xn_nat = xn_nat + db3[None, None, None, :]
    xn = np.ascontiguousarray(xn_nat).astype(NP_BF16)
    # mk[b,p,c,q] = MASK_NEG * (1 - m[b,q,k=c*128+p])  (transposed maskbias)
    mkT = np.float32(MASK_NEG) * (np.float32(1.0) - m.transpose(0, 2, 1))
    mk = np.ascontiguousarray(
        mkT.reshape(nb, NCH, NC_P, BLOCK_LEN).transpose(0, 2, 1, 3)
    ).astype(NP_BF16)

    # w12 = 256 * (W1 W2^T / sqrt(d)); scores = X W12 X^T (zero q/k biases)
    scale = np.float32(C_W12 / math.sqrt(DIM))
    w12 = ((np.asarray(dw1, np.float32) @ np.asarray(dw2, np.float32).T)
           * scale)
    w12 = np.ascontiguousarray(
        w12.reshape(NCH, NC_P, DIM).transpose(1, 0, 2)).astype(NP_F8)
    w3 = np.ascontiguousarray(
        (np.asarray(dw3, np.float32) * np.float32(C_X))
        .reshape(NCH, NC_P, DIM).transpose(1, 0, 2)).astype(NP_F8)

    in_maps = []
    for c in range(N_CORES):
        s = slice(c * NBLK, (c + 1) * NBLK)
        in_maps.append({"xt8": xt8[s], "xtb": xtb[s], "xn": xn[s],
                        "mk": mk[s], "w12": w12, "w3": w3})
    return in_maps


def kernel(inputs, mask_array, dw1, dw2, dw3, db1, db2, db3):
    nc = _get_nc()
    in_maps = prep_in_maps(inputs, mask_array, dw1, dw2, dw3, db1, db2, db3)
    res = run_bass_kernel_spmd(nc, in_maps, list(range(N_CORES)))
    out = np.concatenate(
        [np.asarray(res.results[c]["out"]) for c in range(N_CORES)], axis=0)
    # out[b,p,c,d] -> [b, c*128+p, d]
    out = out.astype(np.float32, copy=False).transpose(0, 2, 1, 3).reshape(
        BATCH, BLOCK_NUM, BLOCK_LEN, DIM)
    return np.ascontiguousarray(out)


# revision 24
# speedup vs baseline: 1.0254x; 1.0254x over previous
"""Block-local attention + LayerNorm kernel for Trainium2 (8 NeuronCores).

Problem (see reference):
  inputs [B=4, bn=16, bl=512, dim=512] fp32
  Q = X@W1, K = X@W2, V = X@W3 (+zero biases)
  S = Q K^T / sqrt(512), masked by elementwise {0,1} mask, softmax over keys
  out = LayerNorm(P @ V + X, eps=1e-3)

Sharding: 64 independent (batch, block) pairs -> 8 blocks per core.

TRN2 measurement: every PE matmul instruction costs ~310 ns regardless
of dtype/mode (fp8 DoubleRow covers K=256 per instr, f32r/bf16 K=128),
so the design minimizes PE instruction count (48/block):
  A^T  : 8  fp8 DoubleRow MMs   (w12 = fp8(256*W1W2^T/sqrt(d)),
                                 xt8 = fp8(X^T/2); PSUM = 128*A^T)
  V    : 8  fp8 DoubleRow MMs   (w3 = fp8(2*W3); PSUM = V exactly)
  S^T  : 16 bf16 MMs            (xtb = bf16(X^T), at = bf16(A^T) via
                                 ACT-copy scale=1/128; PSUM = S^T)
  rowsm: 8  tiny fp8 DR MMs     (r[q] = sum_k P_u^T[k,q], ones rhs)
  O    : 8  fp8 DoubleRow MMs   (P_u^T fp8 x V fp8)
The mask never touches the PE: maskbias^T (0/-240, fp8) is DMA'd
straight into the S^T PSUM tile by a gpsimd casting DMA before the
matmuls accumulate onto it (start=False), so one ACT op per chunk does
exp(S^T + maskbias - 2) -> P_u^T fp8.  Softmax normalization + residual
use LayerNorm scale invariance:
  z = P_u V + r * X  (DVE scalar_tensor_tensor, f32)   LN(z) w/ eps*r^2
so there is no reciprocal or normalize pass over P.
"""

import math
import sys

import numpy as np

sys.path.insert(0, "/opt/trn_rl_repo")

import ml_dtypes

import concourse.bacc as bacc
import concourse.tile as tile
from concourse import masks, mybir
from concourse.bass_utils import run_bass_kernel_spmd

DIM = 512
BLOCK_NUM = 16
SEQ_LEN = 8192
BLOCK_LEN = 512
BATCH = 4
LN_EPS = 1e-3
N_CORES = 8
NBLK = (BATCH * BLOCK_NUM) // N_CORES  # blocks per core
NC_P = 128  # partitions
NCH = DIM // NC_P  # 4 chunks of 128 along dim/token axes

F32 = mybir.dt.float32
BF16 = mybir.dt.bfloat16
F8 = mybir.dt.float8e4
I32 = mybir.dt.int32
DR = mybir.MatmulPerfMode.DoubleRow
EXP = mybir.ActivationFunctionType.Exp
IDENT_FN = mybir.ActivationFunctionType.Identity
MUL = mybir.AluOpType.mult
ADD = mybir.AluOpType.add

NP_F8 = ml_dtypes.float8_e4m3
NP_BF16 = ml_dtypes.bfloat16

MASK_NEG = -240.0  # fp8-max-finite additive mask bias; exp(S-240-2) == 0
EXP_BIAS = -2.0  # headroom shift; cancels exactly in the normalization
C_X = 2.0  # xt8 = X / 2
C_W12 = 256.0  # w12 = 256 * (W1 W2^T/sqrt(d)); A_psum = 128*A
AT_SCALE = 1.0 / 128.0  # at = bf16(A_psum/128) = A


def build_nc(nblk=NBLK, repeat=1):
    nc = bacc.Bacc("TRN2", target_bir_lowering=False, debug=False,
                   num_devices=N_CORES)

    xt8_d = nc.declare_dram_parameter("xt8", [nblk, NC_P, NCH, DIM], F8, isOutput=False)
    xtb_d = nc.declare_dram_parameter("xtb", [nblk, NC_P, NCH, DIM], BF16, isOutput=False)
    xn_d = nc.declare_dram_parameter("xn", [nblk, NC_P, NCH, DIM], BF16, isOutput=False)
    mk_d = nc.declare_dram_parameter("mk", [nblk, NC_P, NCH, DIM], BF16, isOutput=False)
    w12_d = nc.declare_dram_parameter("w12", [NC_P, NCH, DIM], F8, isOutput=False)
    w3_d = nc.declare_dram_parameter("w3", [NC_P, NCH, DIM], F8, isOutput=False)
    out_d = nc.declare_dram_parameter("out", [nblk, NC_P, NCH, DIM], F32, isOutput=True)

    with tile.TileContext(nc) as tc:
        with (
            tc.tile_pool(name="const", bufs=1) as const,
            tc.tile_pool(name="xt8", bufs=3) as p_xt8,
            tc.tile_pool(name="xtb", bufs=2) as p_xtb,
            tc.tile_pool(name="xn", bufs=2) as p_xn,
            tc.tile_pool(name="mk", bufs=2) as p_mk,
            tc.tile_pool(name="at", bufs=2) as p_at,
            tc.tile_pool(name="v", bufs=2) as p_v,
            tc.tile_pool(name="pt", bufs=2) as p_pt,
            tc.tile_pool(name="z", bufs=2) as p_z,
            tc.tile_pool(name="ob", bufs=2) as p_ob,
            tc.tile_pool(name="tiny", bufs=4) as p_tiny,
            tc.tile_pool(name="ps_mm", bufs=3, space="PSUM") as ps_mm,
            tc.tile_pool(name="ps_o", bufs=3, space="PSUM") as ps_o,
            tc.tile_pool(name="ps_r", bufs=1, space="PSUM") as ps_r,
        ):
            # persistent constants
            w12_sb = const.tile([NC_P, NCH, DIM], F8)
            nc.sync.dma_start(out=w12_sb, in_=w12_d[:])
            w3_sb = const.tile([NC_P, NCH, DIM], F8)
            nc.gpsimd.dma_start(out=w3_sb, in_=w3_d[:])
            ones2 = const.tile([NC_P, 2, 1], F8)
            nc.vector.memset(ones2, 1.0)
            ebias = const.tile([NC_P, 1], F32)
            nc.vector.memset(ebias, EXP_BIAS)
            ident = const.tile([NC_P, NC_P], F32)
            masks.make_identity(nc, ident[:])
            identb = const.tile([NC_P, NC_P], BF16)
            nc.vector.tensor_copy(identb[:], ident[:])

            def _blocks():
              for b in range(nblk):
                mk_sb = p_mk.tile([NC_P, NCH, DIM], BF16, tag="mk")
                nc.sync.dma_start(out=mk_sb, in_=mk_d[b])
                xt8_sb = p_xt8.tile([NC_P, NCH, DIM], F8, tag="xt8")
                nc.sync.dma_start(out=xt8_sb, in_=xt8_d[b])
                xtb_sb = p_xtb.tile([NC_P, NCH, DIM], BF16, tag="xtb")
                nc.sync.dma_start(out=xtb_sb, in_=xtb_d[b])
                xn_sb = p_xn.tile([NC_P, NCH, DIM], BF16, tag="xn")
                nc.gpsimd.dma_start(out=xn_sb, in_=xn_d[b])

                # at[d2, t] = bf16(A^T): PSUM = 128*A^T, ACT copy * 1/128
                at_sb = p_at.tile([NC_P, NCH, DIM], BF16, tag="at")
                for d2c in range(NCH):
                    ps = ps_mm.tile([NC_P, DIM], F32, tag="mm")
                    for i in range(2):
                        nc.tensor.matmul(
                            ps[:],
                            lhsT=w12_sb[:, 2 * i:2 * i + 2,
                                        d2c * NC_P:(d2c + 1) * NC_P],
                            rhs=xt8_sb[:, 2 * i:2 * i + 2, :],
                            start=(i == 0), stop=(i == 1), perf_mode=DR)
                    nc.scalar.mul(at_sb[:, d2c, :], ps[:], AT_SCALE)

                # v[t, d'] = fp8(V) = (X^T/2) (2 W3)
                v_sb = p_v.tile([NC_P, NCH, DIM], F8, tag="v")
                for tc_i in range(NCH):
                    ps = ps_mm.tile([NC_P, DIM], F32, tag="mm")
                    for i in range(2):
                        nc.tensor.matmul(
                            ps[:],
                            lhsT=xt8_sb[:, 2 * i:2 * i + 2,
                                        tc_i * NC_P:(tc_i + 1) * NC_P],
                            rhs=w3_sb[:, 2 * i:2 * i + 2, :],
                            start=(i == 0), stop=(i == 1), perf_mode=DR)
                    if tc_i < 2:
                        nc.scalar.copy(v_sb[:, tc_i, :], ps[:])
                    else:
                        nc.vector.tensor_copy(v_sb[:, tc_i, :], ps[:])

                # S^T per key-chunk: PSUM seeded with maskbias^T by a
                # casting DMA, then 4 bf16 MMs accumulate S^T on top;
                # pt = exp(S^T + maskbias - 2) in fp8 (one ACT op each)
                pt_sb = p_pt.tile([NC_P, NCH, DIM], F8, tag="pt")
                for kc in range(NCH):
                    ps = ps_mm.tile([NC_P, DIM], F32, tag="mm")
                    for dc in range(NCH):
                        nc.tensor.matmul(
                            ps[:],
                            lhsT=xtb_sb[:, dc, kc * NC_P:(kc + 1) * NC_P],
                            rhs=at_sb[:, dc, :],
                            start=(dc == 0), stop=False)
                    # mask seeding last (bf16, same PE mode as the S MMs):
                    # S-MMs need not wait for the mk DMA
                    nc.tensor.matmul(ps[:], lhsT=identb[:],
                                     rhs=mk_sb[:, kc, :],
                                     start=False, stop=True)
                    nc.scalar.activation(pt_sb[:, kc, :], ps[:], EXP,
                                         bias=ebias[:])

                # rowsums r[q] = sum_k P_u^T[k, q] via ones-matmuls
                psr = ps_r.tile([NC_P, NCH], F32, tag="r")
                for qc in range(NCH):
                    for i in range(2):
                        nc.tensor.matmul(
                            psr[:, qc:qc + 1],
                            lhsT=pt_sb[:, 2 * i:2 * i + 2,
                                       qc * NC_P:(qc + 1) * NC_P],
                            rhs=ones2[:],
                            start=(i == 0), stop=(i == 1), perf_mode=DR)
                r_sb = p_tiny.tile([NC_P, NCH], F32, tag="r")
                nc.vector.tensor_copy(r_sb[:], psr[:])

                # z = P_u V + r * X  (= r * (attn + X), LN-scale-inv.)
                mvb = p_tiny.tile([NC_P, NCH, 2], F32, tag="mvb")
                z_sb = p_z.tile([NC_P, NCH, DIM], F32, tag="z")
                for qc in range(NCH):
                    pso = ps_o.tile([NC_P, DIM], F32, tag="o")
                    for i in range(2):
                        nc.tensor.matmul(
                            pso[:],
                            lhsT=pt_sb[:, 2 * i:2 * i + 2,
                                       qc * NC_P:(qc + 1) * NC_P],
                            rhs=v_sb[:, 2 * i:2 * i + 2, :],
                            start=(i == 0), stop=(i == 1), perf_mode=DR)
                    nc.vector.scalar_tensor_tensor(
                        out=z_sb[:, qc, :], in0=xn_sb[:, qc, :],
                        scalar=r_sb[:, qc:qc + 1], in1=pso[:],
                        op0=MUL, op1=ADD)
                    stats = p_tiny.tile([NC_P, 6], F32, tag="st")
                    nc.vector.bn_stats(stats[:], z_sb[:, qc, :])
                    nc.vector.bn_aggr(mvb[:, qc, :], stats[:])

                # istd = rsqrt(var_z + eps*r^2), batched over the 4 chunks
                # (magic-constant + 2 Newton steps, DVE only)
                rr = p_tiny.tile([NC_P, NCH], F32, tag="rr")
                nc.vector.tensor_mul(rr[:], r_sb[:], r_sb[:])
                tv = p_tiny.tile([NC_P, NCH], F32, tag="tv")
                nc.vector.scalar_tensor_tensor(
                    out=tv[:], in0=rr[:], scalar=LN_EPS, in1=mvb[:, :, 1],
                    op0=MUL, op1=ADD)
                yv = p_tiny.tile([NC_P, NCH], F32, tag="yv")
                hv = p_tiny.tile([NC_P, NCH], F32, tag="hv")
                nc.vector.tensor_scalar(
                    out=hv[:].bitcast(I32), in0=tv[:].bitcast(I32),
                    scalar1=1, scalar2=None,
                    op0=mybir.AluOpType.logical_shift_right)
                nc.vector.tensor_scalar(
                    out=yv[:].bitcast(I32), in0=hv[:].bitcast(I32),
                    scalar1=-1, scalar2=0x5F3759DF,
                    op0=MUL, op1=ADD)
                av = p_tiny.tile([NC_P, NCH], F32, tag="av")
                cv = p_tiny.tile([NC_P, NCH], F32, tag="cv")
                for _ in range(2):
                    nc.vector.tensor_mul(av[:], yv[:], yv[:])
                    nc.vector.tensor_mul(av[:], av[:], tv[:])
                    nc.vector.tensor_scalar(
                        out=cv[:], in0=av[:], scalar1=-0.5, scalar2=1.5,
                        op0=MUL, op1=ADD)
                    nc.vector.tensor_mul(yv[:], yv[:], cv[:])
                negms = p_tiny.tile([NC_P, NCH], F32, tag="negms")
                nc.vector.tensor_mul(negms[:], mvb[:, :, 0], yv[:])
                nc.vector.tensor_scalar_mul(negms[:], negms[:], -1.0)

                ob_sb = p_ob.tile([NC_P, NCH, DIM], F32, tag="ob")
                for qc in range(NCH):
                    if qc < 2:
                        nc.scalar.activation(
                            ob_sb[:, qc, :], z_sb[:, qc, :], IDENT_FN,
                            bias=negms[:, qc:qc + 1],
                            scale=yv[:, qc:qc + 1])
                    else:
                        nc.vector.tensor_scalar(
                            out=ob_sb[:, qc, :], in0=z_sb[:, qc, :],
                            scalar1=yv[:, qc:qc + 1],
                            scalar2=negms[:, qc:qc + 1],
                            op0=MUL, op1=ADD)
                nc.gpsimd.dma_start(out=out_d[b], in_=ob_sb[:])

            if repeat == 1:
                _blocks()
            else:
                with tc.For_i(0, repeat, 1):
                    _blocks()

    nc.finalize()
    return nc


_NC_CACHE = {}


def _get_nc():
    if "nc" not in _NC_CACHE:
        _NC_CACHE["nc"] = build_nc()
    return _NC_CACHE["nc"]


def prep_in_maps(inputs, mask_array, dw1, dw2, dw3, db1, db2, db3):
    X = np.ascontiguousarray(
        np.asarray(inputs, dtype=np.float32).reshape(
            BATCH * BLOCK_NUM, BLOCK_LEN, DIM))
    m = np.asarray(mask_array, dtype=np.float32).reshape(
        BATCH * BLOCK_NUM, BLOCK_LEN, DIM)
    nb = BATCH * BLOCK_NUM

    # xt[b,p,c,t] = X[b,t,c*128+p]  (X^T in partition-chunk order)
    xt = np.ascontiguousarray(
        X.reshape(nb, BLOCK_LEN, NCH, NC_P).transpose(0, 3, 2, 1))
    xt8 = (xt * np.float32(1.0 / C_X)).astype(NP_F8)
    xtb = xt.astype(NP_BF16)
    # xn[b,p,c,d] = X[b,c*128+p,d]  (bf16, natural rows; b3 folded in)
    xn_nat = X.reshape(nb, NCH, NC_P, DIM).transpose(0, 2, 1, 3)
    db3 = np.asarray(db3, np.float32)
    if db3.any():
        xn_nat = xn_nat + db3[None, None, None, :]
    xn = np.ascontiguousarray(xn_nat).astype(NP_BF16)
    # mk[b,p,c,q] = MASK_NEG * (1 - m[b,q,k=c*128+p])  (transposed maskbias)
    mkT = np.float32(MASK_NEG) * (np.float32(1.0) - m.transpose(0, 2, 1))
    mk = np.ascontiguousarray(
        mkT.reshape(nb, NCH, NC_P, BLOCK_LEN).transpose(0, 2, 1, 3)
    ).astype(NP_BF16)

    # w12 = 256 * (W1 W2^T / sqrt(d)); scores = X W12 X^T (zero q/k biases)
    scale = np.float32(C_W12 / math.sqrt(DIM))
    w12 = ((np.asarray(dw1, np.float32) @ np.asarray(dw2, np.float32).T)
           * scale)
    w12 = np.ascontiguousarray(
        w12.reshape(NCH, NC_P, DIM).transpose(1, 0, 2)).astype(NP_F8)
    w3 = np.ascontiguousarray(
        (np.asarray(dw3, np.float32) * np.float32(C_X))
        .reshape(NCH, NC_P, DIM).transpose(1, 0, 2)).astype(NP_F8)

    in_maps = []
    for c in range(N_CORES):
        s = slice(c * NBLK, (c + 1) * NBLK)
        in_maps.append({"xt8": xt8[s], "xtb": xtb[s], "xn": xn[s],
                        "mk": mk[s], "w12": w12, "w3": w3})
    return in_maps


def kernel(inputs, mask_array, dw1, dw2, dw3, db1, db2, db3):
    nc = _get_nc()
    in_maps = prep_in_maps(inputs, mask_array, dw1, dw2, dw3, db1, db2, db3)
    res = run_bass_kernel_spmd(nc, in_maps, list(range(N_CORES)))
    out = np.concatenate(
        [np.asarray(res.results[c]["out"]) for c in range(N_CORES)], axis=0)
    # out[b,p,c,d] -> [b, c*128+p, d]
    out = out.astype(np.float32, copy=False).transpose(0, 2, 1, 3).reshape(
        BATCH, BLOCK_NUM, BLOCK_LEN, DIM)
    return np.ascontiguousarray(out)


# revision 27
# speedup vs baseline: 1.2974x; 1.2652x over previous
"""Block-local attention + LayerNorm kernel for Trainium2 (8 NeuronCores).

Problem (see reference):
  inputs [B=4, bn=16, bl=512, dim=512] fp32
  Q = X@W1, K = X@W2, V = X@W3 (+zero biases)
  S = Q K^T / sqrt(512), masked by elementwise {0,1} mask, softmax over keys
  out = LayerNorm(P @ V + X, eps=1e-3)

Sharding: 64 independent (batch, block) pairs -> 8 blocks per core.

TRN2 measurement: every PE matmul instruction costs ~310 ns regardless
of dtype/mode (fp8 DoubleRow covers K=256 per instr, f32r/bf16 K=128),
so the design minimizes PE instruction count (48/block):
  A^T  : 8  fp8 DoubleRow MMs   (w12 = fp8(256*W1W2^T/sqrt(d)),
                                 xt8 = fp8(X^T/2); PSUM = 128*A^T)
  V    : 8  fp8 DoubleRow MMs   (w3 = fp8(2*W3); PSUM = V exactly)
  S^T  : 16 bf16 MMs            (xtb = bf16(X^T), at = bf16(A^T) via
                                 ACT-copy scale=1/128; PSUM = S^T)
  rowsm: 8  tiny fp8 DR MMs     (r[q] = sum_k P_u^T[k,q], ones rhs)
  O    : 8  fp8 DoubleRow MMs   (P_u^T fp8 x V fp8)
The mask never touches the PE: maskbias^T (0/-240, fp8) is DMA'd
straight into the S^T PSUM tile by a gpsimd casting DMA before the
matmuls accumulate onto it (start=False), so one ACT op per chunk does
exp(S^T + maskbias - 2) -> P_u^T fp8.  Softmax normalization + residual
use LayerNorm scale invariance:
  z = P_u V + r * X  (DVE scalar_tensor_tensor, f32)   LN(z) w/ eps*r^2
so there is no reciprocal or normalize pass over P.
"""

import math
import sys

import numpy as np

sys.path.insert(0, "/opt/trn_rl_repo")

import ml_dtypes

import concourse.bacc as bacc
import concourse.tile as tile
from concourse import masks, mybir
from concourse.bass_utils import run_bass_kernel_spmd

DIM = 512
BLOCK_NUM = 16
SEQ_LEN = 8192
BLOCK_LEN = 512
BATCH = 4
LN_EPS = 1e-3
N_CORES = 8
NBLK = (BATCH * BLOCK_NUM) // N_CORES  # blocks per core
NC_P = 128  # partitions
NCH = DIM // NC_P  # 4 chunks of 128 along dim/token axes

F32 = mybir.dt.float32
BF16 = mybir.dt.bfloat16
F8 = mybir.dt.float8e4
I32 = mybir.dt.int32
DR = mybir.MatmulPerfMode.DoubleRow
EXP = mybir.ActivationFunctionType.Exp
IDENT_FN = mybir.ActivationFunctionType.Identity
MUL = mybir.AluOpType.mult
ADD = mybir.AluOpType.add

NP_F8 = ml_dtypes.float8_e4m3
NP_BF16 = ml_dtypes.bfloat16

MASK_NEG = -240.0  # fp8-max-finite additive mask bias; exp(S-240-2) == 0
EXP_BIAS = -2.0  # headroom shift; cancels exactly in the normalization
C_X = 2.0  # xt8 = X / 2
C_W12 = 256.0  # w12 = 256 * (W1 W2^T/sqrt(d)); A_psum = 128*A
AT_SCALE = 1.0 / 128.0  # at = bf16(A_psum/128) = A


def build_nc(nblk=NBLK, repeat=1):
    nc = bacc.Bacc("TRN2", target_bir_lowering=False, debug=False,
                   num_devices=N_CORES)

    xt8_d = nc.declare_dram_parameter("xt8", [nblk, NC_P, NCH, DIM], F8, isOutput=False)
    xtb_d = nc.declare_dram_parameter("xtb", [nblk, NC_P, NCH, DIM], BF16, isOutput=False)
    xn_d = nc.declare_dram_parameter("xn", [nblk, NC_P, NCH, DIM], BF16, isOutput=False)
    mk_d = nc.declare_dram_parameter("mk", [nblk, NC_P, NCH, DIM], BF16, isOutput=False)
    w12_d = nc.declare_dram_parameter("w12", [NC_P, NCH, DIM], F8, isOutput=False)
    w3_d = nc.declare_dram_parameter("w3", [NC_P, NCH, DIM], F8, isOutput=False)
    out_d = nc.declare_dram_parameter("out", [nblk, NC_P, NCH, DIM], F32, isOutput=True)

    with tile.TileContext(nc) as tc:
        with (
            tc.tile_pool(name="const", bufs=1) as const,
            tc.tile_pool(name="xt8", bufs=3) as p_xt8,
            tc.tile_pool(name="xtb", bufs=2) as p_xtb,
            tc.tile_pool(name="xn", bufs=2) as p_xn,
            tc.tile_pool(name="mk", bufs=2) as p_mk,
            tc.tile_pool(name="at", bufs=2) as p_at,
            tc.tile_pool(name="v", bufs=2) as p_v,
            tc.tile_pool(name="pt", bufs=2) as p_pt,
            tc.tile_pool(name="sm", bufs=3) as p_sm,
            tc.tile_pool(name="z", bufs=2) as p_z,
            tc.tile_pool(name="ob", bufs=2) as p_ob,
            tc.tile_pool(name="tiny", bufs=4) as p_tiny,
            tc.tile_pool(name="ps_mm", bufs=3, space="PSUM") as ps_mm,
            tc.tile_pool(name="ps_o", bufs=3, space="PSUM") as ps_o,
            tc.tile_pool(name="ps_r", bufs=1, space="PSUM") as ps_r,
        ):
            # persistent constants
            w12_sb = const.tile([NC_P, NCH, DIM], F8)
            nc.sync.dma_start(out=w12_sb, in_=w12_d[:])
            w3_sb = const.tile([NC_P, NCH, DIM], F8)
            nc.gpsimd.dma_start(out=w3_sb, in_=w3_d[:])
            ones2 = const.tile([NC_P, 2, 1], F8)
            nc.vector.memset(ones2, 1.0)
            ebias = const.tile([NC_P, 1], F32)
            nc.vector.memset(ebias, EXP_BIAS)
            ident = const.tile([NC_P, NC_P], F32)
            masks.make_identity(nc, ident[:])
            identb = const.tile([NC_P, NC_P], BF16)
            nc.vector.tensor_copy(identb[:], ident[:])

            def _blocks():
              for b in range(nblk):
                mk_sb = p_mk.tile([NC_P, NCH, DIM], BF16, tag="mk")
                nc.sync.dma_start(out=mk_sb, in_=mk_d[b])
                xt8_sb = p_xt8.tile([NC_P, NCH, DIM], F8, tag="xt8")
                nc.sync.dma_start(out=xt8_sb, in_=xt8_d[b])
                xtb_sb = p_xtb.tile([NC_P, NCH, DIM], BF16, tag="xtb")
                nc.sync.dma_start(out=xtb_sb, in_=xtb_d[b])
                xn_sb = p_xn.tile([NC_P, NCH, DIM], BF16, tag="xn")
                nc.gpsimd.dma_start(out=xn_sb, in_=xn_d[b])

                # at[d2, t] = bf16(A^T): PSUM = 128*A^T, ACT copy * 1/128
                at_sb = p_at.tile([NC_P, NCH, DIM], BF16, tag="at")
                for d2c in range(NCH):
                    ps = ps_mm.tile([NC_P, DIM], F32, tag="mm")
                    for i in range(2):
                        nc.tensor.matmul(
                            ps[:],
                            lhsT=w12_sb[:, 2 * i:2 * i + 2,
                                        d2c * NC_P:(d2c + 1) * NC_P],
                            rhs=xt8_sb[:, 2 * i:2 * i + 2, :],
                            start=(i == 0), stop=(i == 1), perf_mode=DR)
                    nc.scalar.mul(at_sb[:, d2c, :], ps[:], AT_SCALE)

                # v[t, d'] = fp8(V) = (X^T/2) (2 W3)
                v_sb = p_v.tile([NC_P, NCH, DIM], F8, tag="v")
                for tc_i in range(NCH):
                    ps = ps_mm.tile([NC_P, DIM], F32, tag="mm")
                    for i in range(2):
                        nc.tensor.matmul(
                            ps[:],
                            lhsT=xt8_sb[:, 2 * i:2 * i + 2,
                                        tc_i * NC_P:(tc_i + 1) * NC_P],
                            rhs=w3_sb[:, 2 * i:2 * i + 2, :],
                            start=(i == 0), stop=(i == 1), perf_mode=DR)
                    nc.scalar.copy(v_sb[:, tc_i, :], ps[:])

                # S^T per key-chunk: PSUM seeded with maskbias^T by a
                # casting DMA, then 4 bf16 MMs accumulate S^T on top;
                # pt = exp(S^T + maskbias - 2) in fp8 (one ACT op each)
                pt_sb = p_pt.tile([NC_P, NCH, DIM], F8, tag="pt")
                for kc in range(NCH):
                    ps = ps_mm.tile([NC_P, DIM], F32, tag="mm")
                    for dc in range(NCH):
                        nc.tensor.matmul(
                            ps[:],
                            lhsT=xtb_sb[:, dc, kc * NC_P:(kc + 1) * NC_P],
                            rhs=at_sb[:, dc, :],
                            start=(dc == 0), stop=(dc == NCH - 1))
                    # mask add on the drain side (DVE), then fp8 exp (ACT)
                    sm = p_sm.tile([NC_P, DIM], F32, tag="sm")
                    nc.vector.tensor_add(sm[:], ps[:], mk_sb[:, kc, :])
                    nc.scalar.activation(pt_sb[:, kc, :], sm[:], EXP,
                                         bias=ebias[:])

                # rowsums r[q] = sum_k P_u^T[k, q] via ones-matmuls
                psr = ps_r.tile([NC_P, NCH], F32, tag="r")
                for qc in range(NCH):
                    for i in range(2):
                        nc.tensor.matmul(
                            psr[:, qc:qc + 1],
                            lhsT=pt_sb[:, 2 * i:2 * i + 2,
                                       qc * NC_P:(qc + 1) * NC_P],
                            rhs=ones2[:],
                            start=(i == 0), stop=(i == 1), perf_mode=DR)
                r_sb = p_tiny.tile([NC_P, NCH], F32, tag="r")
                nc.vector.tensor_copy(r_sb[:], psr[:])

                # z = P_u V + r * X  (= r * (attn + X), LN-scale-inv.)
                mvb = p_tiny.tile([NC_P, NCH, 2], F32, tag="mvb")
                z_sb = p_z.tile([NC_P, NCH, DIM], F32, tag="z")
                for qc in range(NCH):
                    pso = ps_o.tile([NC_P, DIM], F32, tag="o")
                    for i in range(2):
                        nc.tensor.matmul(
                            pso[:],
                            lhsT=pt_sb[:, 2 * i:2 * i + 2,
                                       qc * NC_P:(qc + 1) * NC_P],
                            rhs=v_sb[:, 2 * i:2 * i + 2, :],
                            start=(i == 0), stop=(i == 1), perf_mode=DR)
                    nc.vector.scalar_tensor_tensor(
                        out=z_sb[:, qc, :], in0=xn_sb[:, qc, :],
                        scalar=r_sb[:, qc:qc + 1], in1=pso[:],
                        op0=MUL, op1=ADD)
                    stats = p_tiny.tile([NC_P, 6], F32, tag="st")
                    nc.vector.bn_stats(stats[:], z_sb[:, qc, :])
                    nc.vector.bn_aggr(mvb[:, qc, :], stats[:])

                # istd = rsqrt(var_z + eps*r^2), batched over the 4 chunks
                # (magic-constant + 2 Newton steps, DVE only)
                rr = p_tiny.tile([NC_P, NCH], F32, tag="rr")
                nc.vector.tensor_mul(rr[:], r_sb[:], r_sb[:])
                tv = p_tiny.tile([NC_P, NCH], F32, tag="tv")
                nc.vector.scalar_tensor_tensor(
                    out=tv[:], in0=rr[:], scalar=LN_EPS, in1=mvb[:, :, 1],
                    op0=MUL, op1=ADD)
                yv = p_tiny.tile([NC_P, NCH], F32, tag="yv")
                hv = p_tiny.tile([NC_P, NCH], F32, tag="hv")
                nc.vector.tensor_scalar(
                    out=hv[:].bitcast(I32), in0=tv[:].bitcast(I32),
                    scalar1=1, scalar2=None,
                    op0=mybir.AluOpType.logical_shift_right)
                nc.vector.tensor_scalar(
                    out=yv[:].bitcast(I32), in0=hv[:].bitcast(I32),
                    scalar1=-1, scalar2=0x5F3759DF,
                    op0=MUL, op1=ADD)
                av = p_tiny.tile([NC_P, NCH], F32, tag="av")
                cv = p_tiny.tile([NC_P, NCH], F32, tag="cv")
                for _ in range(2):
                    nc.vector.tensor_mul(av[:], yv[:], yv[:])
                    nc.vector.tensor_mul(av[:], av[:], tv[:])
                    nc.vector.tensor_scalar(
                        out=cv[:], in0=av[:], scalar1=-0.5, scalar2=1.5,
                        op0=MUL, op1=ADD)
                    nc.vector.tensor_mul(yv[:], yv[:], cv[:])
                negms = p_tiny.tile([NC_P, NCH], F32, tag="negms")
                nc.vector.tensor_mul(negms[:], mvb[:, :, 0], yv[:])
                nc.vector.tensor_scalar_mul(negms[:], negms[:], -1.0)

                ob_sb = p_ob.tile([NC_P, NCH, DIM], F32, tag="ob")
                for qc in range(NCH):
                    if qc < 2:
                        nc.scalar.activation(
                            ob_sb[:, qc, :], z_sb[:, qc, :], IDENT_FN,
                            bias=negms[:, qc:qc + 1],
                            scale=yv[:, qc:qc + 1])
                    else:
                        nc.vector.tensor_scalar(
                            out=ob_sb[:, qc, :], in0=z_sb[:, qc, :],
                            scalar1=yv[:, qc:qc + 1],
                            scalar2=negms[:, qc:qc + 1],
                            op0=MUL, op1=ADD)
                nc.gpsimd.dma_start(out=out_d[b], in_=ob_sb[:])

            if repeat == 1:
                _blocks()
            else:
                with tc.For_i(0, repeat, 1):
                    _blocks()

    nc.finalize()
    return nc


_NC_CACHE = {}


def _get_nc():
    if "nc" not in _NC_CACHE:
        _NC_CACHE["nc"] = build_nc()
    return _NC_CACHE["nc"]


def prep_in_maps(inputs, mask_array, dw1, dw2, dw3, db1, db2, db3):
    X = np.ascontiguousarray(
        np.asarray(inputs, dtype=np.float32).reshape(
            BATCH * BLOCK_NUM, BLOCK_LEN, DIM))
    m = np.asarray(mask_array, dtype=np.float32).reshape(
        BATCH * BLOCK_NUM, BLOCK_LEN, DIM)
    nb = BATCH * BLOCK_NUM

    # xt[b,p,c,t] = X[b,t,c*128+p]  (X^T in partition-chunk order)
    xt = np.ascontiguousarray(
        X.reshape(nb, BLOCK_LEN, NCH, NC_P).transpose(0, 3, 2, 1))
    xt8 = (xt * np.float32(1.0 / C_X)).astype(NP_F8)
    xtb = xt.astype(NP_BF16)
    # xn[b,p,c,d] = X[b,c*128+p,d]  (bf16, natural rows; b3 folded in)
    xn_nat = X.reshape(nb, NCH, NC_P, DIM).transpose(0, 2, 1, 3)
    db3 = np.asarray(db3, np.float32)
    if db3.any():
        xn_nat = xn_nat + db3[None, None, None, :]
    xn = np.ascontiguousarray(xn_nat).astype(NP_BF16)
    # mk[b,p,c,q] = MASK_NEG * (1 - m[b,q,k=c*128+p])  (transposed maskbias)
    mkT = np.float32(MASK_NEG) * (np.float32(1.0) - m.transpose(0, 2, 1))
    mk = np.ascontiguousarray(
        mkT.reshape(nb, NCH, NC_P, BLOCK_LEN).transpose(0, 2, 1, 3)
    ).astype(NP_BF16)

    # w12 = 256 * (W1 W2^T / sqrt(d)); scores = X W12 X^T (zero q/k biases)
    scale = np.float32(C_W12 / math.sqrt(DIM))
    w12 = ((np.asarray(dw1, np.float32) @ np.asarray(dw2, np.float32).T)
           * scale)
    w12 = np.ascontiguousarray(
        w12.reshape(NCH, NC_P, DIM).transpose(1, 0, 2)).astype(NP_F8)
    w3 = np.ascontiguousarray(
        (np.asarray(dw3, np.float32) * np.float32(C_X))
        .reshape(NCH, NC_P, DIM).transpose(1, 0, 2)).astype(NP_F8)

    in_maps = []
    for c in range(N_CORES):
        s = slice(c * NBLK, (c + 1) * NBLK)
        in_maps.append({"xt8": xt8[s], "xtb": xtb[s], "xn": xn[s],
                        "mk": mk[s], "w12": w12, "w3": w3})
    return in_maps


def kernel(inputs, mask_array, dw1, dw2, dw3, db1, db2, db3):
    nc = _get_nc()
    in_maps = prep_in_maps(inputs, mask_array, dw1, dw2, dw3, db1, db2, db3)
    res = run_bass_kernel_spmd(nc, in_maps, list(range(N_CORES)))
    out = np.concatenate(
        [np.asarray(res.results[c]["out"]) for c in range(N_CORES)], axis=0)
    # out[b,p,c,d] -> [b, c*128+p, d]
    out = out.astype(np.float32, copy=False).transpose(0, 2, 1, 3).reshape(
        BATCH, BLOCK_NUM, BLOCK_LEN, DIM)
    return np.ascontiguousarray(out)
